# revision 4
# baseline (speedup 1.0000x reference)
"""Trainium2 Bass kernel for CompoundProteinInteractionPrediction.

Contract: kernel(**inputs) takes the FULL (unsharded) numpy inputs of
reference.setup_inputs() and returns the full output (shape [2], float32).

Strategy (8 NeuronCores, one chip):
  - GNN phase: row-shard the adjacency matmul. Core c owns atom rows
    [c*1024, (c+1)*1024). The host uploads A[rows_c, :].T as fp32; the device
    streams it once, casting to a resident bf16 A^T tile in SBUF that all
    three GNN layers reuse (A is 0/1 so bf16 is exact). Per layer:
    hs^T = relu(Wg_i @ xs^T + bg) locally for own atoms, AllGather hs (bf16),
    DMA-transpose into natural [atoms, dim] layout, then accumulate
    (A_shard @ hs)^T over 64 k-chunks into PSUM and add into the fp32 xs^T.
  - x_compound: per-core column-sum of final xs^T, AllGather + local reduce.
  - Protein phase: shard the 8192 positions; conv(23x23) is computed as 23
    accumulating matmuls against banded Toeplitz matrices (host-built);
    per-layer AllGather of the reweighted xs_p (bf16) with a padded DRAM
    image + dynamic-offset halo load (partition_id register).
  - Head: logits = [x_c; x_p] @ Wo^T + bo, stable softmax, on every core;
    host reads core 0's output.
"""

import numpy as np

import concourse.bacc as bacc
import concourse.bass as bass
import concourse.mybir as mybir
import concourse.tile as tile
from concourse import bass_utils
from concourse.bass import ds
from concourse.masks import make_identity

F32 = mybir.dt.float32
BF16 = mybir.dt.bfloat16
I32 = mybir.dt.int32

W = 8            # cores
NA = 8192        # atoms
D = 128          # dim
R = NA // W      # atom rows per core (1024)
KC = NA // 128   # contraction chunks (64)
LG = 3           # GNN layers
LC = 3           # CNN layers
WIN = 11
KK = 2 * WIN + 1  # 23
LP = 8192        # protein length
RP = LP // W     # protein positions per core (1024)
NFP = 100000
NWORD = 10000

RG = [list(range(W))]  # replica groups


def _build():
    nc = bacc.Bacc("TRN2", target_bir_lowering=False, debug=False, num_devices=W)

    # ---- per-core external inputs ----
    at_in = nc.dram_tensor("at_shard", [NA, R], F32, kind="ExternalInput")
    fp_idx_in = nc.dram_tensor("fp_idx", [128, NA // 128 // W], I32, kind="ExternalInput")
    wd_idx_in = nc.dram_tensor("wd_idx", [128, LP // 128 // W], I32, kind="ExternalInput")
    emb_fp_in = nc.dram_tensor("emb_fp", [NFP, D], F32, kind="ExternalInput")
    emb_word_in = nc.dram_tensor("emb_word", [NWORD, D], F32, kind="ExternalInput")
    wg_in = nc.dram_tensor("wg_t", [128, LG * 128], F32, kind="ExternalInput")
    bg_in = nc.dram_tensor("bg_t", [128, LG], F32, kind="ExternalInput")
    wa_in = nc.dram_tensor("wa_t", [128, 128], F32, kind="ExternalInput")
    ba_in = nc.dram_tensor("ba_t", [128, 1], F32, kind="ExternalInput")
    cm_in = nc.dram_tensor("conv_m", [128, LC * KK * 128], F32, kind="ExternalInput")
    cb_in = nc.dram_tensor("cb_t", [128, LC], F32, kind="ExternalInput")
    wo_in = nc.dram_tensor("wo_t", [128, 4], F32, kind="ExternalInput")
    bo_in = nc.dram_tensor("bo_t", [1, 2], F32, kind="ExternalInput")

    z_out = nc.dram_tensor("z", [1, 2], F32, kind="ExternalOutput")
    dbg_xc = nc.dram_tensor("dbg_xc", [128, 1], F32, kind="ExternalOutput")
    dbg_xp = nc.dram_tensor("dbg_xp", [128, 1], F32, kind="ExternalOutput")
    dbg_logits = nc.dram_tensor("dbg_logits", [1, 2], F32, kind="ExternalOutput")

    # ---- internal DRAM (collective bounce buffers) ----
    hs_bin = [nc.dram_tensor(f"hs_bin{i}", [128, R], BF16) for i in range(LG)]
    hs_gath = [nc.dram_tensor(f"hs_gath{i}", [W * 128, R], BF16, addr_space="Shared")
               for i in range(LG)]
    pa_bin = nc.dram_tensor("pa_bin", [128, 1], F32)
    pa_gath = nc.dram_tensor("pa_gath", [W * 128, 1], F32, addr_space="Shared")
    xsp_bin = [nc.dram_tensor(f"xsp_bin{l}", [128, RP], BF16) for l in range(LC)]
    xsp_gath = [nc.dram_tensor(f"xsp_gath{l}", [W * 128, RP], BF16, addr_space="Shared")
                for l in range(LC)]
    pp_bin = nc.dram_tensor("pp_bin", [128, 1], F32)
    pp_gath = nc.dram_tensor("pp_gath", [W * 128, 1], F32, addr_space="Shared")
    xsp_pad = nc.dram_tensor("xsp_pad", [128, LP + 2 * WIN], BF16)

    with tile.TileContext(nc) as tc:
        with (
            tc.tile_pool(name="const", bufs=1) as cpool,
            tc.tile_pool(name="psum", bufs=1, space="PSUM") as ppool,
            tc.tile_pool(name="stage", bufs=3) as spool,
        ):
            # ---------- constants / weights ----------
            ident = cpool.tile([128, 128], F32)
            make_identity(nc, ident[:])

            wg_f = cpool.tile([128, LG * 128], F32)
            nc.sync.dma_start(wg_f[:], wg_in[:])
            wg_bf = cpool.tile([128, LG * 128], BF16)
            nc.vector.tensor_copy(wg_bf[:], wg_f[:])

            wa_f = cpool.tile([128, 128], F32)
            nc.sync.dma_start(wa_f[:], wa_in[:])
            wa_bf = cpool.tile([128, 128], BF16)
            nc.vector.tensor_copy(wa_bf[:], wa_f[:])

            wo_f = cpool.tile([128, 4], F32)
            nc.sync.dma_start(wo_f[:], wo_in[:])
            wo_bf = cpool.tile([128, 4], BF16)
            nc.vector.tensor_copy(wo_bf[:], wo_f[:])

            bg_f = cpool.tile([128, LG], F32)
            nc.sync.dma_start(bg_f[:], bg_in[:])
            ba_f = cpool.tile([128, 1], F32)
            nc.sync.dma_start(ba_f[:], ba_in[:])
            cb_f = cpool.tile([128, LC], F32)
            nc.sync.dma_start(cb_f[:], cb_in[:])
            bo_f = cpool.tile([1, 2], F32)
            nc.sync.dma_start(bo_f[:], bo_in[:])

            ones_bf = cpool.tile([1, 128], BF16)
            nc.gpsimd.memset(ones_bf[:], 1.0)

            fp_idx = cpool.tile([128, NA // 128 // W], I32)
            nc.sync.dma_start(fp_idx[:], fp_idx_in[:])
            wd_idx = cpool.tile([128, LP // 128 // W], I32)
            nc.sync.dma_start(wd_idx[:], wd_idx_in[:])

            # ---------- protein embedding gather (own slice) + early AllGather ----------
            xspT_bf = spool.tile([128, RP], BF16, tag="xspT0", bufs=1)
            for j in range(RP // 128):
                xg = spool.tile([128, D], F32, tag="xg")
                nc.gpsimd.indirect_dma_start(
                    out=xg[:], out_offset=None, in_=emb_word_in[:],
                    in_offset=bass.IndirectOffsetOnAxis(ap=wd_idx[:, j:j + 1], axis=0),
                )
                pt = ppool.tile([128, 128], F32, tag="b4", name="pt")
                nc.tensor.transpose(out=pt[:], in_=xg[:], identity=ident[:])
                nc.scalar.copy(xspT_bf[:, j * 128:(j + 1) * 128], pt[:])
            nc.sync.dma_start(xsp_bin[0][:], xspT_bf[:])
            nc.gpsimd.collective_compute(
                "AllGather", mybir.AluOpType.bypass, replica_groups=RG,
                ins=[xsp_bin[0][:].opt()], outs=[xsp_gath[0][:].opt()],
            )

            # ---------- compound embedding gather (own atoms) ----------
            xsT = cpool.tile([128, R], F32)  # fp32 xs^T accumulator, own atoms
            for j in range(R // 128):
                xg = spool.tile([128, D], F32, tag="xg")
                nc.gpsimd.indirect_dma_start(
                    out=xg[:], out_offset=None, in_=emb_fp_in[:],
                    in_offset=bass.IndirectOffsetOnAxis(ap=fp_idx[:, j:j + 1], axis=0),
                )
                pt = ppool.tile([128, 128], F32, tag="b4", name="pt")
                nc.tensor.transpose(out=pt[:], in_=xg[:], identity=ident[:])
                nc.vector.tensor_copy(xsT[:, j * 128:(j + 1) * 128], pt[:])

            # ---------- phase A: GNN ----------
            with tc.tile_pool(name="aresident", bufs=1) as apool:
                at_bf = apool.tile([128, KC, R], BF16)  # resident A^T shard (bf16)
                # stream fp32 A^T shard, cast to bf16 (read once, reuse 3x)
                for k in range(KC):
                    stg = spool.tile([128, R], F32, tag="atstg")
                    nc.sync.dma_start(stg[:], at_in[k * 128:(k + 1) * 128, :])
                    eng = nc.vector if (k % 2 == 0) else nc.scalar
                    if k % 2 == 0:
                        nc.vector.tensor_copy(at_bf[:, k, :], stg[:])
                    else:
                        nc.scalar.copy(at_bf[:, k, :], stg[:])

                for i in range(LG):
                    # hs^T own = relu(Wg_i @ xs^T + bg_i), cast bf16
                    xsT_bf = spool.tile([128, R], BF16, tag="xsbf")
                    nc.vector.tensor_copy(xsT_bf[:], xsT[:])
                    hsT_bf = spool.tile([128, R], BF16, tag="hsT")
                    for h in range(2):
                        ph = ppool.tile([128, 512], F32, tag=f"b{h}", name="ph")
                        nc.tensor.matmul(
                            ph[:], lhsT=wg_bf[:, i * 128:(i + 1) * 128],
                            rhs=xsT_bf[:, h * 512:(h + 1) * 512],
                            start=True, stop=True,
                        )
                        nc.scalar.activation(
                            hsT_bf[:, h * 512:(h + 1) * 512], ph[:],
                            mybir.ActivationFunctionType.Relu,
                            bias=bg_f[:, i:i + 1],
                        )
                    nc.sync.dma_start(hs_bin[i][:], hsT_bf[:])
                    nc.gpsimd.collective_compute(
                        "AllGather", mybir.AluOpType.bypass, replica_groups=RG,
                        ins=[hs_bin[i][:].opt()], outs=[hs_gath[i][:].opt()],
                    )
                    # transpose gathered hs^T blocks into natural [atoms, d] chunks
                    hs_nat = spool.tile([128, KC, 128], BF16, tag="hsnat", bufs=1)
                    for cb in range(W):
                        nc.sync.dma_start_transpose(
                            out=hs_nat[:, cb * (KC // W):(cb + 1) * (KC // W), :],
                            in_=hs_gath[i][cb * 128:(cb + 1) * 128, :],
                        )
                    # upd^T = (A_shard @ hs)^T accumulated over k chunks
                    pu = [ppool.tile([128, 512], F32, tag=f"b{2+h}", name=f"pu{h}") for h in range(2)]
                    for k in range(KC):
                        for h in range(2):
                            nc.tensor.matmul(
                                pu[h][:], lhsT=hs_nat[:, k, :],
                                rhs=at_bf[:, k, h * 512:(h + 1) * 512],
                                start=(k == 0), stop=(k == KC - 1),
                            )
                    for h in range(2):
                        nc.vector.tensor_add(
                            xsT[:, h * 512:(h + 1) * 512],
                            xsT[:, h * 512:(h + 1) * 512], pu[h][:],
                        )

            # x_compound partial: column-sum of own xs^T slice, then AllGather
            partial = cpool.tile([128, 1], F32)
            nc.vector.tensor_reduce(partial[:], xsT[:], axis=mybir.AxisListType.X,
                                    op=mybir.AluOpType.add)
            nc.sync.dma_start(pa_bin[:], partial[:])
            nc.gpsimd.collective_compute(
                "AllGather", mybir.AluOpType.bypass, replica_groups=RG,
                ins=[pa_bin[:].opt()], outs=[pa_gath[:].opt()],
            )
            pa_sb = cpool.tile([128, W], F32)
            nc.sync.dma_start(pa_sb[:], pa_gath[:].rearrange("(c p) o -> p c o", c=W))
            xcT = cpool.tile([128, 1], F32)
            nc.vector.tensor_reduce(xcT[:], pa_sb[:], axis=mybir.AxisListType.X,
                                    op=mybir.AluOpType.add)
            nc.sync.dma_start(dbg_xc[:], xcT[:])

            # ---------- phase B: attention-CNN over protein ----------
            with tc.tile_pool(name="bpool", bufs=2) as bpool:
                # conv Toeplitz weights -> bf16
                cm_bf = cpool.tile([128, LC * KK, 128], BF16)
                NW_CH = 23
                for t in range(0, LC * KK, NW_CH):
                    n = min(NW_CH, LC * KK - t)
                    stg = bpool.tile([128, NW_CH * 128], F32, tag="cmstg")
                    nc.sync.dma_start(stg[:, :n * 128], cm_in[:, t * 128:(t + n) * 128])
                    nc.vector.tensor_copy(
                        cm_bf[:, t:t + n, :],
                        stg[:, :n * 128].rearrange("p (a d) -> p a d", d=128))

                # zero the padded-image margins once
                zt = bpool.tile([128, WIN], BF16, tag="zt")
                nc.gpsimd.memset(zt[:], 0.0)
                nc.sync.dma_start(xsp_pad[:, 0:WIN], zt[:])
                nc.sync.dma_start(xsp_pad[:, WIN + LP:WIN + LP + WIN], zt[:])

                pid = nc.sync.partition_id()  # core id register (sync engine)

                x_cur = bpool.tile([128, 1], F32, tag="xcur")
                nc.vector.tensor_copy(x_cur[:], xcT[:])

                xpT = cpool.tile([128, 1], F32)

                for l in range(LC):
                    # place gathered xs_p blocks into padded DRAM image
                    for cb in range(W):
                        nc.sync.dma_start(
                            xsp_pad[:, WIN + cb * RP:WIN + (cb + 1) * RP],
                            xsp_gath[l][cb * 128:(cb + 1) * 128, :])
                    # halo load of own window (dynamic offset by core id)
                    xs_loc = bpool.tile([128, RP + 2 * WIN], BF16, tag="xsloc")
                    nc.sync.dma_start(xs_loc[:], xsp_pad[:, ds(pid * RP, RP + 2 * WIN)])

                    # conv as 23 accumulating Toeplitz matmuls
                    hs1_bf = bpool.tile([128, RP], BF16, tag="hs1")
                    for h in range(2):
                        pc = ppool.tile([128, 512], F32, tag=f"b{5+h}", name="pc")
                        for a in range(KK):
                            nc.tensor.matmul(
                                pc[:], lhsT=cm_bf[:, l * KK + a, :],
                                rhs=xs_loc[:, h * 512 + a:h * 512 + a + 512],
                                start=(a == 0), stop=(a == KK - 1),
                            )
                        nc.scalar.activation(
                            hs1_bf[:, h * 512:(h + 1) * 512], pc[:],
                            mybir.ActivationFunctionType.Relu,
                            bias=cb_f[:, l:l + 1],
                        )

                    # hs2 = relu(Wa @ hs1 + ba)
                    hs2_f = bpool.tile([128, RP], F32, tag="hs2f")
                    hs2_bf = bpool.tile([128, RP], BF16, tag="hs2b")
                    for h in range(2):
                        p2 = ppool.tile([128, 512], F32, tag=f"b{5+h}", name="p2")
                        nc.tensor.matmul(p2[:], lhsT=wa_bf[:],
                                         rhs=hs1_bf[:, h * 512:(h + 1) * 512],
                                         start=True, stop=True)
                        nc.scalar.activation(
                            hs2_f[:, h * 512:(h + 1) * 512], p2[:],
                            mybir.ActivationFunctionType.Relu, bias=ba_f[:, 0:1])
                    nc.vector.tensor_copy(hs2_bf[:], hs2_f[:])

                    # x = relu(Wa @ x + ba)
                    xb = bpool.tile([128, 1], BF16, tag="xb")
                    nc.vector.tensor_copy(xb[:], x_cur[:])
                    px = ppool.tile([128, 1], F32, tag="b4", name="px")
                    nc.tensor.matmul(px[:], lhsT=wa_bf[:], rhs=xb[:],
                                     start=True, stop=True)
                    x_cur = bpool.tile([128, 1], F32, tag="xcur")
                    nc.scalar.activation(x_cur[:], px[:],
                                         mybir.ActivationFunctionType.Relu,
                                         bias=ba_f[:, 0:1])
                    xb2 = bpool.tile([128, 1], BF16, tag="xb2")
                    nc.vector.tensor_copy(xb2[:], x_cur[:])

                    # w = tanh(x . hs2), broadcast to 128 partitions via K=1 matmul
                    w_bf = bpool.tile([1, RP], BF16, tag="wbf")
                    for h in range(2):
                        pw = ppool.tile([1, 512], F32, tag=f"b{5+h}", name="pw")
                        nc.tensor.matmul(pw[:], lhsT=xb2[:],
                                         rhs=hs2_bf[:, h * 512:(h + 1) * 512],
                                         start=True, stop=True)
                        nc.scalar.activation(w_bf[:, h * 512:(h + 1) * 512], pw[:],
                                             mybir.ActivationFunctionType.Tanh)

                    last = l == LC - 1
                    xspn_bf = bpool.tile([128, RP], BF16, tag="xspn")
                    xspn_f = bpool.tile([128, RP], F32, tag="xspnf", name="xspn_f") if last else None
                    for h in range(2):
                        pb = ppool.tile([128, 512], F32, tag=f"b{5+h}", name="pb")
                        nc.tensor.matmul(pb[:], lhsT=ones_bf[:],
                                         rhs=w_bf[:, h * 512:(h + 1) * 512],
                                         start=True, stop=True)
                        if last:
                            nc.vector.tensor_mul(xspn_f[:, h * 512:(h + 1) * 512],
                                                 pb[:], hs2_f[:, h * 512:(h + 1) * 512])
                        else:
                            nc.vector.tensor_mul(xspn_bf[:, h * 512:(h + 1) * 512],
                                                 pb[:], hs2_f[:, h * 512:(h + 1) * 512])
                    if not last:
                        nc.sync.dma_start(xsp_bin[l + 1][:], xspn_bf[:])
                        nc.gpsimd.collective_compute(
                            "AllGather", mybir.AluOpType.bypass, replica_groups=RG,
                            ins=[xsp_bin[l + 1][:].opt()],
                            outs=[xsp_gath[l + 1][:].opt()],
                        )
                    else:
                        partp = bpool.tile([128, 1], F32, tag="partp")
                        nc.vector.tensor_reduce(partp[:], xspn_f[:],
                                                axis=mybir.AxisListType.X,
                                                op=mybir.AluOpType.add)
                        nc.sync.dma_start(pp_bin[:], partp[:])
                        nc.gpsimd.collective_compute(
                            "AllGather", mybir.AluOpType.bypass, replica_groups=RG,
                            ins=[pp_bin[:].opt()], outs=[pp_gath[:].opt()],
                        )
                        pp_sb = bpool.tile([128, W], F32, tag="ppsb")
                        nc.sync.dma_start(
                            pp_sb[:], pp_gath[:].rearrange("(c p) o -> p c o", c=W))
                        nc.vector.tensor_reduce(xpT[:], pp_sb[:],
                                                axis=mybir.AxisListType.X,
                                                op=mybir.AluOpType.add)
                nc.sync.dma_start(dbg_xp[:], xpT[:])

                # ---------- head ----------
                xc_bf = bpool.tile([128, 1], BF16, tag="xcbf")
                nc.vector.tensor_copy(xc_bf[:], xcT[:])
                xp_bf = bpool.tile([128, 1], BF16, tag="xpbf")
                nc.vector.tensor_copy(xp_bf[:], xpT[:])
                pz = ppool.tile([1, 2], F32, tag="b4", name="pz")
                nc.tensor.matmul(pz[:], lhsT=xc_bf[:], rhs=wo_bf[:, 0:2],
                                 start=True, stop=False)
                nc.tensor.matmul(pz[:], lhsT=xp_bf[:], rhs=wo_bf[:, 2:4],
                                 start=False, stop=True)
                lg_t = bpool.tile([1, 2], F32, tag="lg")
                nc.vector.tensor_add(lg_t[:], pz[:], bo_f[:])
                nc.sync.dma_start(dbg_logits[:], lg_t[:])
                mx = bpool.tile([1, 1], F32, tag="mx")
                nc.vector.tensor_reduce(mx[:], lg_t[:], axis=mybir.AxisListType.X,
                                        op=mybir.AluOpType.max)
                nmx = bpool.tile([1, 1], F32, tag="nmx")
                nc.scalar.mul(nmx[:], mx[:], -1.0)
                ex = bpool.tile([1, 2], F32, tag="ex")
                nc.scalar.activation(ex[:], lg_t[:],
                                     mybir.ActivationFunctionType.Exp,
                                     bias=nmx[:, 0:1])
                sm = bpool.tile([1, 1], F32, tag="sm")
                nc.vector.tensor_reduce(sm[:], ex[:], axis=mybir.AxisListType.X,
                                        op=mybir.AluOpType.add)
                rc = bpool.tile([1, 1], F32, tag="rc")
                nc.vector.reciprocal(rc[:], sm[:])
                zsb = bpool.tile([1, 2], F32, tag="zsb")
                nc.vector.tensor_scalar_mul(zsb[:], ex[:], rc[:, 0:1])
                nc.sync.dma_start(z_out[:], zsb[:])

    nc.compile()
    return nc


_NC_CACHE = {}


def _get_nc():
    if "nc" not in _NC_CACHE:
        _NC_CACHE["nc"] = _build()
    return _NC_CACHE["nc"]


def _prep_host(inputs):
    fingerprints = np.asarray(inputs["fingerprints"]).astype(np.int32)
    words = np.asarray(inputs["words"]).astype(np.int32)
    adjacency = np.ascontiguousarray(np.asarray(inputs["adjacency"], dtype=np.float32))
    emb_fp = np.ascontiguousarray(np.asarray(inputs["emb_fp"], dtype=np.float32))
    emb_word = np.ascontiguousarray(np.asarray(inputs["emb_word"], dtype=np.float32))
    Wg = np.asarray(inputs["Wg"], dtype=np.float32)
    bg = np.asarray(inputs["bg"], dtype=np.float32)
    conv_w = np.asarray(inputs["conv_w"], dtype=np.float32)
    conv_b = np.asarray(inputs["conv_b"], dtype=np.float32)
    Wa = np.asarray(inputs["Wa"], dtype=np.float32)
    ba = np.asarray(inputs["ba"], dtype=np.float32)
    Wo = np.asarray(inputs["Wo"], dtype=np.float32)
    bo = np.asarray(inputs["bo"], dtype=np.float32)

    wg_t = np.ascontiguousarray(np.concatenate([Wg[i].T for i in range(LG)], axis=1))
    bg_t = np.ascontiguousarray(bg.T)                      # [128, LG]
    wa_t = np.ascontiguousarray(Wa.T)                      # [128, 128]
    ba_t = np.ascontiguousarray(ba.reshape(128, 1))

    # banded Toeplitz matrices M[l, a][d, d'] = conv_w[l,0,0,a, d-d'+11]
    dd = np.arange(128)
    band = dd[:, None] - dd[None, :] + WIN                 # [128, 128]
    valid = (band >= 0) & (band <= KK - 1)
    bandc = np.clip(band, 0, KK - 1)
    cm = np.zeros((128, LC * KK * 128), dtype=np.float32)
    for l in range(LC):
        for a in range(KK):
            M = np.where(valid, conv_w[l, 0, 0, a][bandc], 0.0).astype(np.float32)
            cm[:, (l * KK + a) * 128:(l * KK + a + 1) * 128] = M
    cb_t = np.ascontiguousarray(np.tile(conv_b[None, :], (128, 1)))
    wo_t = np.ascontiguousarray(
        np.concatenate([Wo[:, :128].T, Wo[:, 128:].T], axis=1))  # [128, 4]
    bo_t = np.ascontiguousarray(bo.reshape(1, 2))

    shared = {
        "emb_fp": emb_fp, "emb_word": emb_word,
        "wg_t": wg_t, "bg_t": bg_t, "wa_t": wa_t, "ba_t": ba_t,
        "conv_m": cm, "cb_t": cb_t, "wo_t": wo_t, "bo_t": bo_t,
    }
    in_maps = []
    for c in range(W):
        at_shard = np.ascontiguousarray(adjacency[c * R:(c + 1) * R, :].T)
        fp_c = np.ascontiguousarray(
            fingerprints[c * R:(c + 1) * R].reshape(R // 128, 128).T)
        wd_c = np.ascontiguousarray(
            words[c * RP:(c + 1) * RP].reshape(RP // 128, 128).T)
        m = dict(shared)
        m["at_shard"] = at_shard
        m["fp_idx"] = fp_c
        m["wd_idx"] = wd_c
        in_maps.append(m)
    return in_maps


def run_raw(inputs):
    """Build + run; returns the full per-core results (for debugging)."""
    nc = _get_nc()
    in_maps = _prep_host(inputs)
    res = bass_utils.run_bass_kernel_spmd(nc, in_maps, core_ids=list(range(W)))
    return res


def kernel(**inputs) -> np.ndarray:
    res = run_raw(inputs)
    return np.ascontiguousarray(res.results[0]["z"].reshape(2).astype(np.float32))
